# revision 19
# baseline (speedup 1.0000x reference)
"""GCN/GCDE message-passing kernel for 8 Trainium2 NeuronCores (v2).

out = softplus(norm * (A @ (norm * x)) @ W + bias),  norm = rsqrt(max(deg,1)) (0 if deg==0)

Strategy (dst-sharded graph parallel, streaming halo; evolution of v1):
  - 8-way shard by destination node; host buckets edges, degree-sorts dst
    nodes into 128-row chunks, groups 4 chunks per PSUM accumulator, and
    stages the halo (src feature rows per edge slot) as dense payload.
  - Payload is fp8 e3m4 (half the DMA bytes of f16; ~4x the precision of
    e4m3) laid out FEATURE-OUTER per piece: [128 part, 64 feat, S tiles,
    4 chunks], so the per-edge src-norm multiply has a packed innermost
    axis (slots) and the broadcast norm axis (feat) is outer -> DVE runs
    in 2x mode for f16 operands.
  - The fp8->f16 convert+scale work is split across three engines to
    balance load: Pool (gpsimd) does fp8*norm->f16 directly, ACT converts
    fp8->f16 then DVE multiplies at 2x, and DVE multiplies fp8 directly
    at 1x. Weighted round-robin per sub-chunk of 4 tiles.
  - Aggregation is PSUM-accumulated identity matmuls (f16). Multiple
    tiles ride in one matmul instruction via a broadcast (stride-0) PSUM
    out AP -- the revisited columns accumulate, cutting LDWEIGHTS and
    instruction count by MT x.
  - Epilogue per group: dst-norm multiply into (chunk, feat)-ordered f16,
    DMA-crossbar transposes (no PE/ACT involvement), W matmuls into one
    [64, 512] PSUM bank, batched softplus (exp then ln on ACT), one
    output DMA per group.
  - Src nodes with zero in-degree have their payload zeroed by the host
    (index work), so no deg>0 masks are needed on device; staged degrees
    are pre-clamped to >= 1.

Host side does integer/index work and dtype conversion only (bucketing,
degree counting, sorting, padding, row duplication/zeroing); all FP
arithmetic runs on the NeuronCores.
"""

import sys
from contextlib import ExitStack

sys.path.insert(0, "/opt/trn_rl_repo")

import numpy as np

import concourse.bacc as bacc
import concourse.mybir as mybir
from concourse.masks import make_identity
from concourse.tile import TileContext

F32 = mybir.dt.float32
F16 = mybir.dt.float16
F8E3 = mybir.dt.float8e3

ALU = mybir.AluOpType
ACTF = mybir.ActivationFunctionType


def _r128(v):
    return (v + 127) // 128 * 128


class Geom:
    def __init__(self, n_nodes=50000, n_cores=8, d=64, cpg=4, s=32, cc=4,
                 payload="f8e3", split=(0.0, 0.55, 0.45), mt=1,
                 dma_transpose=False, pool_mode="cp8"):
        assert n_nodes % n_cores == 0
        self.N = n_nodes
        self.D = d
        self.CORES = n_cores
        self.NSH = n_nodes // n_cores
        self.CH = _r128(self.NSH) // 128  # 128-dst chunks per core
        self.CPG = cpg
        self.GG = (self.CH + cpg - 1) // cpg
        self.CHPAD = self.GG * cpg
        self.SLOTS = self.CHPAD * 128
        self.S = s    # max tiles per DMA piece
        self.CC = cc  # tiles per convert/scale op
        self.payload = payload  # "f8e3" | "f16"
        self.split = split      # (pool, act, dve) work fractions
        self.MT = mt            # tiles per matmul instruction (1 = off)
        self.dma_transpose = dma_transpose
        self.pool_mode = pool_mode  # "tt8" | "cp8" | "tt16"


def _np_payload_dtype(geom):
    if geom.payload == "f8e3":
        import ml_dtypes
        return ml_dtypes.float8_e3m4
    return np.float16


def _bir_payload_dtype(geom):
    return F8E3 if geom.payload == "f8e3" else F16


def _rank_within_group(keys):
    order = np.argsort(keys, kind="stable")
    sk = keys[order]
    starts = np.r_[0, np.flatnonzero(sk[1:] != sk[:-1]) + 1]
    grp = np.zeros(len(keys), dtype=np.int64)
    grp[starts] = 1
    grp = np.cumsum(grp) - 1
    ranks_sorted = np.arange(len(keys)) - starts[grp]
    ranks = np.empty(len(keys), dtype=np.int64)
    ranks[order] = ranks_sorted
    return ranks


def make_plan(src, dst, geom):
    """Host-side integer work: bucket edges per core, degree-sort dst nodes,
    build the slot->src mapping and the global tile schedule TG."""
    g = geom
    deg_full = np.bincount(dst, minlength=g.N).astype(np.int64)

    cores = []
    Tc = np.zeros((g.CORES, g.GG), dtype=np.int64)
    for c in range(g.CORES):
        lo = c * g.NSH
        m = (dst >= lo) & (dst < lo + g.NSH)
        es, ed = src[m], dst[m] - lo
        deg = np.bincount(ed, minlength=g.NSH)
        perm = np.argsort(-deg, kind="stable")  # local ids, degree desc
        slot_of = np.empty(g.NSH, dtype=np.int64)
        slot_of[perm] = np.arange(g.NSH)
        degsorted = np.zeros(g.GG * g.CPG * 128, dtype=np.int64)
        degsorted[: g.NSH] = deg[perm]
        Tc[c] = degsorted.reshape(g.GG, g.CPG * 128).max(axis=1)
        cores.append(dict(es=es, ed=ed, perm=perm, slot_of=slot_of))

    TG = np.maximum(Tc.max(axis=0), 1)  # global (all cores share the schedule)
    baseG = np.r_[0, np.cumsum(TG)][:-1]
    TOT = int(TG.sum())

    # DMA pieces: per group, runs of <= S tiles; last piece may be partial.
    pieces = []  # list per group of (tglobal0, SQ, dram_off_elems_per_part)
    off = 0
    for gg in range(g.GG):
        T = int(TG[gg])
        pl = []
        t0 = 0
        while t0 < T:
            SQ = min(g.S, T - t0)
            pl.append((int(baseG[gg]) + t0, SQ, off))
            off += 64 * SQ * g.CPG
            t0 += SQ
        pieces.append(pl)
    total_elems = off

    plans = []
    for c in range(g.CORES):
        w = cores[c]
        slots = w["slot_of"][w["ed"]]
        t = _rank_within_group(w["ed"])
        gg = slots // (g.CPG * 128)
        j4 = (slots // 128) % g.CPG
        p = slots % 128
        rows = (baseG[gg] + t) * 128 + p
        plans.append(
            dict(rows=rows, j4=j4, es=w["es"], perm=w["perm"])
        )
    return dict(TG=TG, baseG=baseG, TOT=TOT, plans=plans, deg_full=deg_full,
                pieces=pieces, total_elems=total_elems)


def _patch_act_tables():
    import concourse.bacc as _bacc

    if getattr(_bacc, "_gcde_tables_patched", False):
        return
    orig = _bacc.get_activation_tables

    def patched(arch):
        tabs = orig(arch)
        keep = "natural_log_exp_and_others"
        if keep in tabs:
            for k in list(tabs.keys()):
                if k != keep:
                    tabs[k] = set()
        return tabs

    _bacc.get_activation_tables = patched
    _bacc._gcde_tables_patched = True


def _engine_pattern(split, n=64):
    """Deterministic weighted round-robin over ('pool','act','dve')."""
    w = dict(zip(("pool", "act", "dve"), split))
    issued = dict.fromkeys(w, 0)
    pat = []
    for k in range(1, n + 1):
        e = max(w, key=lambda e: w[e] * k - issued[e])
        issued[e] += 1
        pat.append(e)
    return pat


def build_nc(geom, plan):
    _patch_act_tables()
    g = geom
    TG = plan["TG"]
    TOT = plan["TOT"]
    CW = g.CPG * g.D
    xdt = _bir_payload_dtype(g)
    nc = bacc.Bacc("TRN2", target_bir_lowering=False, debug=False)

    xg_d = nc.dram_tensor("xg", [128, plan["total_elems"]], xdt, kind="ExternalInput")
    degg_d = nc.dram_tensor("degg", [128, TOT * g.CPG], F16, kind="ExternalInput")
    degA_d = nc.dram_tensor("degA", [128, g.CHPAD], F32, kind="ExternalInput")
    # block-diag(W, W) so one K=128 matmul transforms a chunk pair
    wblk_d = nc.dram_tensor("wblk", [128, 128], F16, kind="ExternalInput")
    bias2_d = nc.dram_tensor("bias2", [128, 1], F32, kind="ExternalInput")
    # rows (chunk%2, feat); cols (group, pair, dst) -- host untangles
    outT_d = nc.dram_tensor("outT", [128, g.GG * g.CPG * 64], F32,
                            kind="ExternalOutput")

    pattern = _engine_pattern(g.split)
    pat_i = 0

    with TileContext(nc) as tc, ExitStack() as _st:
        const = _st.enter_context(tc.tile_pool(name="const", bufs=1))
        xp = _st.enter_context(tc.tile_pool(name="xp", bufs=4))
        sp = _st.enter_context(tc.tile_pool(name="sp", bufs=4))
        psG = _st.enter_context(tc.tile_pool(name="psG", bufs=3, space="PSUM"))
        psT = _st.enter_context(tc.tile_pool(name="psT", bufs=1, space="PSUM"))
        small = _st.enter_context(tc.tile_pool(name="small", bufs=4))

        ident = const.tile([128, 128], F32)
        make_identity(nc, ident)
        ident16 = const.tile([128, 128], F16, tag="ident16")
        nc.vector.tensor_copy(ident16[:], ident[:])
        wblk = const.tile([128, 128], F16, tag="wblk")
        nc.sync.dma_start(wblk[:], wblk_d[:, :])
        bias2 = const.tile([128, 1], F32, tag="bias2")
        nc.sync.dma_start(bias2[:], bias2_d[:, :])

        # dst-side norm: rsqrt(deg) with deg pre-clamped >= 1 by the host
        degA_sb = const.tile([128, g.CHPAD], F32)
        nc.sync.dma_start(degA_sb[:], degA_d[:, :])
        lnA = const.tile([128, g.CHPAD], F32, tag="lnA")
        normA = const.tile([128, g.CHPAD], F32, tag="normA")
        nc.scalar.activation(lnA[:], degA_sb[:], ACTF.Ln)
        nc.scalar.activation(normA[:], lnA[:], ACTF.Exp, scale=-0.5)

        # src-side norm per slot (deg pre-clamped >= 1; zero-in-degree srcs
        # have zeroed payload instead of a mask)
        degg_sb = const.tile([128, TOT * g.CPG], F16, tag="degg")
        nc.sync.dma_start(degg_sb[:], degg_d[:, :])
        lng = const.tile([128, TOT * g.CPG], F16, tag="lng")
        normg = const.tile([128, TOT * g.CPG], F16, tag="normg")
        nc.scalar.activation(lng[:], degg_sb[:], ACTF.Ln)
        nc.scalar.activation(normg[:], lng[:], ACTF.Exp, scale=-0.5)
        normg_v = normg[:].rearrange("p (t j) -> p t j", j=g.CPG)

        def epilogue(gg, ps):
            # dst-norm multiply, reorder (f,j) -> (j,f), f32 -> f16
            vG = small.tile([128, g.CPG, g.D], F16, tag="vG")
            ps_jf = ps[:].rearrange("p (f j) -> p j f", j=g.CPG)
            nAb = normA[:, gg * g.CPG : (gg + 1) * g.CPG, None].broadcast_to(
                [128, g.CPG, g.D]
            )
            nc.vector.tensor_tensor(vG[:], ps_jf, nAb, ALU.mult)

            # transpose chunk pairs: [128 dst, 2*64 feat] -> [(2,64) feat, 128 dst]
            # then one block-diag W matmul per pair; softplus; store
            npr = g.CPG // 2
            pO = psT.tile([128, npr * 128], F32, tag="pO")
            for pr in range(npr):
                aT = small.tile([128, 128], F16, tag=f"aT{pr}")
                if g.dma_transpose:
                    nc.sync.dma_start_transpose(aT[:], vG[:, 2 * pr : 2 * pr + 2, :])
                else:
                    pT = psT.tile([128, 128], F16, tag=f"pT{pr}")
                    nc.tensor.matmul(pT[:], vG[:, 2 * pr : 2 * pr + 2, :],
                                     ident16[:], is_transpose=True)
                    nc.scalar.copy(aT[:], pT[:])
                nc.tensor.matmul(pO[:, pr * 128 : (pr + 1) * 128], wblk[:], aT[:],
                                 start=True, stop=True)

            # softplus(z + bias) = ln(1 + exp(z + bias)); rows = (chunk%2, feat)
            ez = small.tile([128, npr * 128], F32, tag="ez")
            nc.scalar.activation(ez[:], pO[:], ACTF.Exp, bias=bias2[:])
            ob = small.tile([128, npr * 128], F32, tag="ob")
            nc.scalar.activation(ob[:], ez[:], ACTF.Ln, bias=1.0)
            nc.sync.dma_start(
                outT_d[:, gg * npr * 128 : (gg + 1) * npr * 128], ob[:]
            )

        pending = []  # (gg, ps) epilogues deferred one group for overlap
        for gg in range(g.GG):
            T = int(TG[gg])
            ps = psG.tile([128, CW], F32, tag="ps")
            npieces = len(plan["pieces"][gg])
            for qi, (tg0, SQ, off) in enumerate(plan["pieces"][gg]):
                ne = 64 * SQ * g.CPG
                xt = xp.tile([128, g.S * g.D * g.CPG], xdt, tag="xt")
                nc.sync.dma_start(xt[:, :ne], xg_d[:, off : off + ne])
                xs = sp.tile([128, g.S * g.D * g.CPG], F16, tag="xs")
                xt_v = xt[:, :ne].rearrange("p (f t j) -> p f t j", f=g.D, j=g.CPG)
                xs_v = xs[:, :ne].rearrange("p (f t j) -> p f t j", f=g.D, j=g.CPG)

                # convert + src-norm multiply, split across engines
                c0 = 0
                while c0 < SQ:
                    CCq = min(g.CC, SQ - c0)
                    nbc = normg_v[:, None, tg0 + c0 : tg0 + c0 + CCq, :].broadcast_to(
                        [128, g.D, CCq, g.CPG]
                    )
                    o = xs_v[:, :, c0 : c0 + CCq, :]
                    i = xt_v[:, :, c0 : c0 + CCq, :]
                    eng = pattern[pat_i % len(pattern)]
                    pat_i += 1
                    if g.payload != "f8e3":
                        eng = "dve"  # f16 payload: DVE 2x handles everything
                    if eng == "pool":
                        if g.pool_mode == "tt8":
                            nc.gpsimd.tensor_tensor(o, i, nbc, ALU.mult)
                        elif g.pool_mode == "cp8":
                            nc.gpsimd.tensor_copy(o, i)
                            nc.vector.tensor_tensor(o, o, nbc, ALU.mult)
                        else:  # tt16: ACT converts, Pool multiplies
                            nc.scalar.copy(o, i)
                            nc.gpsimd.tensor_tensor(o, o, nbc, ALU.mult)
                    elif eng == "act":
                        nc.scalar.copy(o, i)
                        nc.vector.tensor_tensor(o, o, nbc, ALU.mult)
                    else:
                        nc.vector.tensor_tensor(o, i, nbc, ALU.mult)
                    c0 += CCq

                # aggregate: identity matmuls, MT tiles per instruction
                xs_t = xs[:, :ne].rearrange("p (f t j) -> p t f j", f=g.D, j=g.CPG)
                t0 = 0
                while t0 < SQ:
                    MTq = min(g.MT, SQ - t0)
                    first = qi == 0 and t0 == 0
                    last = qi == npieces - 1 and t0 + MTq == SQ
                    if MTq == 1:
                        rhs = xs_t[:, t0, :, :]
                        out_ap = ps[:]
                    else:
                        rhs = xs_t[:, t0 : t0 + MTq, :, :]
                        out_ap = ps[:, None, :].broadcast_to([128, MTq, CW])
                    nc.tensor.matmul(out_ap, ident16[:], rhs,
                                     start=first, stop=last)
                    t0 += MTq

            # defer this group's epilogue until after the next group's
            # payload work, so the DVE/PE queues never stall on psum
            pending.append((gg, ps))
            if len(pending) > 1:
                epilogue(*pending.pop(0))
        for e in pending:
            epilogue(*e)

    nc.compile()
    return nc


def _in_maps(x, weight, bias, geom, plan):
    g = geom
    x = np.ascontiguousarray(np.asarray(x, dtype=np.float32))
    deg_full = plan["deg_full"]
    xdt = _np_payload_dtype(g)
    xq = x.astype(xdt)
    xq[deg_full == 0] = 0  # src-side norm is 0 for zero-in-degree nodes
    degmax = np.maximum(deg_full, 1).astype(np.float16)

    TOT = plan["TOT"]
    w16 = np.asarray(weight, dtype=np.float32).astype(np.float16)
    wblk = np.zeros((128, 128), dtype=np.float16)
    wblk[: g.D, : g.D] = w16
    wblk[g.D :, g.D :] = w16
    bias2 = np.tile(np.asarray(bias, dtype=np.float32).reshape(g.D, 1), (2, 1))
    base = {
        "wblk": np.ascontiguousarray(wblk),
        "bias2": np.ascontiguousarray(bias2),
    }
    maps = []
    for c in range(g.CORES):
        p = plan["plans"][c]
        A = np.zeros((TOT * 128, g.CPG, g.D), dtype=xdt)
        A[p["rows"], p["j4"]] = xq[p["es"]]
        D2 = np.ones((TOT * 128, g.CPG), dtype=np.float16)
        D2[p["rows"], p["j4"]] = degmax[p["es"]]
        degA = np.ones(g.SLOTS, dtype=np.float32)
        degA[: g.NSH] = np.maximum(deg_full[c * g.NSH + p["perm"]], 1)

        # feature-outer piece-major payload: [128, f, t, j] per piece
        F = A.reshape(TOT, 128, g.CPG, g.D).transpose(1, 3, 0, 2)  # p f t j
        blocks = []
        for gl in plan["pieces"]:
            for (tg0, SQ, off) in gl:
                blocks.append(
                    np.ascontiguousarray(F[:, :, tg0 : tg0 + SQ, :]).reshape(128, -1)
                )
        xg = np.concatenate(blocks, axis=1)
        assert xg.shape[1] == plan["total_elems"]

        degg_pm = np.ascontiguousarray(
            D2.reshape(TOT, 128, g.CPG).transpose(1, 0, 2).reshape(128, -1)
        )
        maps.append(
            dict(
                base,
                xg=xg,
                degg=degg_pm,
                degA=np.ascontiguousarray(degA.reshape(g.CHPAD, 128).T),
            )
        )
    return maps


def _unshard(outTs, geom, plan):
    g = geom
    out = np.empty((g.N, g.D), dtype=np.float32)
    for c in range(g.CORES):
        perm = plan["plans"][c]["perm"]
        # outT rows (chunk%2, feat); cols (group, pair, dst)
        O = outTs[c].reshape(2, g.D, g.GG, g.CPG // 2, 128)
        C = O.transpose(2, 3, 0, 1, 4).reshape(g.CHPAD, g.D, 128)
        full = C.transpose(1, 0, 2).reshape(g.D, g.CHPAD * 128)
        out[c * g.NSH + perm] = full[:, : g.NSH].T
    return out


def run_sim(inputs, geom):
    from concourse.bass_interp import MultiCoreSim

    plan = make_plan(np.asarray(inputs["src"]), np.asarray(inputs["dst"]), geom)
    nc = build_nc(geom, plan)
    maps = _in_maps(inputs["x"], inputs["weight"], inputs["bias"], geom, plan)
    sim = MultiCoreSim(nc, num_cores=geom.CORES, trace=False)
    cores = list(sim.cores.values())
    for c, core in enumerate(cores):
        for name, arr in maps[c].items():
            core.tensor(name)[:] = arr
    sim.simulate(check_with_hw=False)
    outTs = [np.array(core.tensor("outT")) for core in cores]
    return _unshard(outTs, geom, plan)


def _install_ntff_hook():
    """The agent image's antenv lacks axon_hooks; recreate the ctypes NTFF
    profile hook (mirrors trn_agent_boot) so trace=True yields exec times."""
    import contextlib
    import ctypes
    import types

    import antenv

    if "antenv.axon_hooks" in sys.modules:
        return
    lib = ctypes.CDLL("/opt/axon/libaxon_pjrt.so")
    if not hasattr(lib, "axon_start_nrt_profile"):
        return
    lib.axon_start_nrt_profile.argtypes = [ctypes.POINTER(ctypes.c_int64), ctypes.c_size_t]
    lib.axon_start_nrt_profile.restype = ctypes.c_int64
    lib.axon_stop_nrt_profile.argtypes = [ctypes.c_char_p]
    lib.axon_stop_nrt_profile.restype = ctypes.c_int64

    @contextlib.contextmanager
    def _hook(output_dir, device_ids):
        import jax

        jax.devices()
        if device_ids:
            ids = (ctypes.c_int64 * len(device_ids))(*device_ids)
            rc = lib.axon_start_nrt_profile(ids, len(device_ids))
        else:
            rc = lib.axon_start_nrt_profile(None, 0)
        if rc != 0:
            raise RuntimeError(f"axon_start_nrt_profile rc={rc}")
        try:
            yield
        finally:
            n = lib.axon_stop_nrt_profile(str(output_dir).encode())
            print(f"ntff profile: {n} file(s) -> {output_dir}", file=sys.stderr)

    mod = types.ModuleType("antenv.axon_hooks")
    mod._hook = _hook
    mod.get_axon_ntff_profile_hook = lambda: _hook
    mod.set_axon_ntff_profile_hook = lambda h: None
    sys.modules["antenv.axon_hooks"] = mod
    antenv.axon_hooks = mod


def run_hw(inputs, geom, trace=False):
    from concourse.bass_utils import run_bass_kernel_spmd

    if trace:
        import concourse.bass_utils as _bu

        _install_ntff_hook()
        _bu.upload_artifacts = lambda d: "local://" + str(d)

    plan = make_plan(np.asarray(inputs["src"]), np.asarray(inputs["dst"]), geom)
    nc = build_nc(geom, plan)
    maps = _in_maps(inputs["x"], inputs["weight"], inputs["bias"], geom, plan)
    import tempfile

    tdir = tempfile.mkdtemp(prefix="gcde_trace_") if trace else None
    res = run_bass_kernel_spmd(
        nc, maps, core_ids=list(range(geom.CORES)), trace=trace, tmpdir=tdir
    )
    if trace:
        print("trace dir:", tdir, file=sys.stderr)
    outTs = [r["outT"] for r in res.results]
    out = _unshard(outTs, geom, plan)
    return out, res


def kernel(**inputs):
    geom = Geom(n_nodes=50000, n_cores=8)
    out, _ = run_hw(inputs, geom)
    return out


# revision 24
# speedup vs baseline: 1.7931x; 1.7931x over previous
"""GCN/GCDE message-passing kernel for 8 Trainium2 NeuronCores (v2).

out = softplus(norm * (A @ (norm * x)) @ W + bias),  norm = rsqrt(max(deg,1)) (0 if deg==0)

Strategy (dst-sharded graph parallel, streaming halo; evolution of v1):
  - 8-way shard by destination node; host buckets edges, degree-sorts dst
    nodes into 128-row chunks, groups 4 chunks per PSUM accumulator, and
    stages the halo (src feature rows per edge slot) as dense payload.
  - Payload is fp8 e3m4 (half the DMA bytes of f16; ~4x the precision of
    e4m3) laid out FEATURE-OUTER per piece: [128 part, 64 feat, S tiles,
    4 chunks], so the per-edge src-norm multiply has a packed innermost
    axis (slots) and the broadcast norm axis (feat) is outer -> DVE runs
    in 2x mode for f16 operands.
  - The fp8->f16 convert+scale work is split across three engines to
    balance load: Pool (gpsimd) does fp8*norm->f16 directly, ACT converts
    fp8->f16 then DVE multiplies at 2x, and DVE multiplies fp8 directly
    at 1x. Weighted round-robin per sub-chunk of 4 tiles.
  - Aggregation is PSUM-accumulated identity matmuls (f16). Multiple
    tiles ride in one matmul instruction via a broadcast (stride-0) PSUM
    out AP -- the revisited columns accumulate, cutting LDWEIGHTS and
    instruction count by MT x.
  - Epilogue per group: dst-norm multiply into (chunk, feat)-ordered f16,
    DMA-crossbar transposes (no PE/ACT involvement), W matmuls into one
    [64, 512] PSUM bank, batched softplus (exp then ln on ACT), one
    output DMA per group.
  - Src nodes with zero in-degree have their payload zeroed by the host
    (index work), so no deg>0 masks are needed on device; staged degrees
    are pre-clamped to >= 1.

Host side does integer/index work and dtype conversion only (bucketing,
degree counting, sorting, padding, row duplication/zeroing); all FP
arithmetic runs on the NeuronCores.
"""

import sys
from contextlib import ExitStack

sys.path.insert(0, "/opt/trn_rl_repo")

import numpy as np

import concourse.bacc as bacc
import concourse.mybir as mybir
from concourse.masks import make_identity
from concourse.tile import TileContext

F32 = mybir.dt.float32
F16 = mybir.dt.float16
F8E3 = mybir.dt.float8e3

ALU = mybir.AluOpType
ACTF = mybir.ActivationFunctionType


def _r128(v):
    return (v + 127) // 128 * 128


class Geom:
    def __init__(self, n_nodes=50000, n_cores=8, d=64, cpg=4, s=32, cc=4,
                 payload="f8e3", split=(0.0, 0.55, 0.45), mt=1,
                 dma_transpose=False, pool_mode="cp8"):
        assert n_nodes % n_cores == 0
        self.N = n_nodes
        self.D = d
        self.CORES = n_cores
        self.NSH = n_nodes // n_cores
        self.CH = _r128(self.NSH) // 128  # 128-dst chunks per core
        self.CPG = cpg
        self.GG = (self.CH + cpg - 1) // cpg
        self.CHPAD = self.GG * cpg
        self.SLOTS = self.CHPAD * 128
        self.S = s    # max tiles per DMA piece
        self.CC = cc  # tiles per convert/scale op
        self.payload = payload  # "f8e3" | "f16"
        self.split = split      # (pool, act, dve) work fractions
        self.MT = mt            # tiles per matmul instruction (1 = off)
        self.dma_transpose = dma_transpose
        self.pool_mode = pool_mode  # "tt8" | "cp8" | "tt16"


def _np_payload_dtype(geom):
    if geom.payload == "f8e3":
        import ml_dtypes
        return ml_dtypes.float8_e3m4
    return np.float16


def _bir_payload_dtype(geom):
    return F8E3 if geom.payload == "f8e3" else F16


def _rank_within_group(keys):
    order = np.argsort(keys, kind="stable")
    sk = keys[order]
    starts = np.r_[0, np.flatnonzero(sk[1:] != sk[:-1]) + 1]
    grp = np.zeros(len(keys), dtype=np.int64)
    grp[starts] = 1
    grp = np.cumsum(grp) - 1
    ranks_sorted = np.arange(len(keys)) - starts[grp]
    ranks = np.empty(len(keys), dtype=np.int64)
    ranks[order] = ranks_sorted
    return ranks


def make_plan(src, dst, geom):
    """Host-side integer work: bucket edges per core, degree-sort dst nodes,
    build the slot->src mapping and the global tile schedule TG."""
    g = geom
    deg_full = np.bincount(dst, minlength=g.N).astype(np.int64)

    cores = []
    Tc = np.zeros((g.CORES, g.GG), dtype=np.int64)
    for c in range(g.CORES):
        lo = c * g.NSH
        m = (dst >= lo) & (dst < lo + g.NSH)
        es, ed = src[m], dst[m] - lo
        deg = np.bincount(ed, minlength=g.NSH)
        perm = np.argsort(-deg, kind="stable")  # local ids, degree desc
        slot_of = np.empty(g.NSH, dtype=np.int64)
        slot_of[perm] = np.arange(g.NSH)
        degsorted = np.zeros(g.GG * g.CPG * 128, dtype=np.int64)
        degsorted[: g.NSH] = deg[perm]
        Tc[c] = degsorted.reshape(g.GG, g.CPG * 128).max(axis=1)
        cores.append(dict(es=es, ed=ed, perm=perm, slot_of=slot_of))

    TG = np.maximum(Tc.max(axis=0), 1)  # global (all cores share the schedule)
    baseG = np.r_[0, np.cumsum(TG)][:-1]
    TOT = int(TG.sum())

    # DMA pieces: per group, runs of <= S tiles; last piece may be partial.
    pieces = []  # list per group of (tglobal0, SQ, dram_off_elems_per_part)
    off = 0
    for gg in range(g.GG):
        T = int(TG[gg])
        pl = []
        t0 = 0
        while t0 < T:
            SQ = min(g.S, T - t0)
            pl.append((int(baseG[gg]) + t0, SQ, off))
            off += 64 * SQ * g.CPG
            t0 += SQ
        pieces.append(pl)
    total_elems = off

    plans = []
    for c in range(g.CORES):
        w = cores[c]
        slots = w["slot_of"][w["ed"]]
        t = _rank_within_group(w["ed"])
        gg = slots // (g.CPG * 128)
        j4 = (slots // 128) % g.CPG
        p = slots % 128
        rows = (baseG[gg] + t) * 128 + p
        plans.append(
            dict(rows=rows, j4=j4, es=w["es"], perm=w["perm"])
        )
    return dict(TG=TG, baseG=baseG, TOT=TOT, plans=plans, deg_full=deg_full,
                pieces=pieces, total_elems=total_elems)


def _patch_act_tables():
    import concourse.bacc as _bacc

    if getattr(_bacc, "_gcde_tables_patched", False):
        return
    orig = _bacc.get_activation_tables

    def patched(arch):
        tabs = orig(arch)
        keep = "natural_log_exp_and_others"
        if keep in tabs:
            for k in list(tabs.keys()):
                if k != keep:
                    tabs[k] = set()
        return tabs

    _bacc.get_activation_tables = patched
    _bacc._gcde_tables_patched = True


def _engine_pattern(split, n=64):
    """Deterministic weighted round-robin over ('pool','act','dve')."""
    w = dict(zip(("pool", "act", "dve"), split))
    issued = dict.fromkeys(w, 0)
    pat = []
    for k in range(1, n + 1):
        e = max(w, key=lambda e: w[e] * k - issued[e])
        issued[e] += 1
        pat.append(e)
    return pat


def build_nc(geom, plan):
    _patch_act_tables()
    g = geom
    TG = plan["TG"]
    TOT = plan["TOT"]
    CW = g.CPG * g.D
    xdt = _bir_payload_dtype(g)
    nc = bacc.Bacc("TRN2", target_bir_lowering=False, debug=False)

    xg_d = nc.dram_tensor("xg", [128, plan["total_elems"]], xdt, kind="ExternalInput")
    degg_d = nc.dram_tensor("degg", [128, TOT * g.CPG], F16, kind="ExternalInput")
    degA_d = nc.dram_tensor("degA", [128, g.CHPAD], F32, kind="ExternalInput")
    # block-diag(W, W) so one K=128 matmul transforms a chunk pair
    wblk_d = nc.dram_tensor("wblk", [128, 128], F16, kind="ExternalInput")
    bias2_d = nc.dram_tensor("bias2", [128, 1], F32, kind="ExternalInput")
    # rows (chunk%2, feat); cols (group, pair, dst) -- host untangles
    outT_d = nc.dram_tensor("outT", [128, g.GG * g.CPG * 64], F32,
                            kind="ExternalOutput")

    pattern = _engine_pattern(g.split)
    pat_i = 0

    with TileContext(nc) as tc, ExitStack() as _st:
        const = _st.enter_context(tc.tile_pool(name="const", bufs=1))
        xp = _st.enter_context(tc.tile_pool(name="xp", bufs=4))
        sp = _st.enter_context(tc.tile_pool(name="sp", bufs=4))
        psG = _st.enter_context(tc.tile_pool(name="psG", bufs=3, space="PSUM"))
        psT = _st.enter_context(tc.tile_pool(name="psT", bufs=1, space="PSUM"))
        small = _st.enter_context(tc.tile_pool(name="small", bufs=4))

        ident = const.tile([128, 128], F32)
        make_identity(nc, ident)
        ident16 = const.tile([128, 128], F16, tag="ident16")
        nc.vector.tensor_copy(ident16[:], ident[:])
        wblk = const.tile([128, 128], F16, tag="wblk")
        nc.sync.dma_start(wblk[:], wblk_d[:, :])
        bias2 = const.tile([128, 1], F32, tag="bias2")
        nc.sync.dma_start(bias2[:], bias2_d[:, :])

        # dst-side norm: rsqrt(deg) with deg pre-clamped >= 1 by the host
        degA_sb = const.tile([128, g.CHPAD], F32)
        nc.sync.dma_start(degA_sb[:], degA_d[:, :])
        lnA = const.tile([128, g.CHPAD], F32, tag="lnA")
        normA = const.tile([128, g.CHPAD], F32, tag="normA")
        nc.scalar.activation(lnA[:], degA_sb[:], ACTF.Ln)
        nc.scalar.activation(normA[:], lnA[:], ACTF.Exp, scale=-0.5)

        # src-side norm per slot (deg pre-clamped >= 1; zero-in-degree srcs
        # have zeroed payload instead of a mask). Computed per group so the
        # first payload multiply isn't blocked on the full-length prep.
        degg_sb = const.tile([128, TOT * g.CPG], F16, tag="degg")
        nc.sync.dma_start(degg_sb[:], degg_d[:, :])
        lng = const.tile([128, TOT * g.CPG], F16, tag="lng")
        normg = const.tile([128, TOT * g.CPG], F16, tag="normg")

        def prep_normg(gg):
            b0 = int(plan["baseG"][gg]) * g.CPG
            b1 = (int(plan["baseG"][gg]) + int(TG[gg])) * g.CPG
            nc.scalar.activation(lng[:, b0:b1], degg_sb[:, b0:b1], ACTF.Ln)
            nc.scalar.activation(normg[:, b0:b1], lng[:, b0:b1], ACTF.Exp,
                                 scale=-0.5)

        prep_normg(0)
        normg_v = normg[:].rearrange("p (t j) -> p t j", j=g.CPG)

        def epilogue(gg, ps):
            # dst-norm multiply, reorder (f,j) -> (j,f), f32 -> f16
            vG = small.tile([128, g.CPG, g.D], F16, tag="vG")
            ps_jf = ps[:].rearrange("p (f j) -> p j f", j=g.CPG)
            nAb = normA[:, gg * g.CPG : (gg + 1) * g.CPG, None].broadcast_to(
                [128, g.CPG, g.D]
            )
            nc.vector.tensor_tensor(vG[:], ps_jf, nAb, ALU.mult)

            # transpose chunk pairs: [128 dst, 2*64 feat] -> [(2,64) feat, 128 dst]
            # then one block-diag W matmul per pair; softplus; store
            npr = g.CPG // 2
            pO = psT.tile([128, npr * 128], F32, tag="pO")
            for pr in range(npr):
                aT = small.tile([128, 128], F16, tag=f"aT{pr}")
                if g.dma_transpose:
                    nc.sync.dma_start_transpose(aT[:], vG[:, 2 * pr : 2 * pr + 2, :])
                else:
                    pT = psT.tile([128, 128], F16, tag=f"pT{pr}")
                    nc.tensor.matmul(pT[:], vG[:, 2 * pr : 2 * pr + 2, :],
                                     ident16[:], is_transpose=True)
                    nc.scalar.copy(aT[:], pT[:])
                nc.tensor.matmul(pO[:, pr * 128 : (pr + 1) * 128], wblk[:], aT[:],
                                 start=True, stop=True)

            # softplus(z + bias) = ln(1 + exp(z + bias)); rows = (chunk%2, feat)
            ez = small.tile([128, npr * 128], F32, tag="ez")
            nc.scalar.activation(ez[:], pO[:], ACTF.Exp, bias=bias2[:])
            ob = small.tile([128, npr * 128], F32, tag="ob")
            nc.scalar.activation(ob[:], ez[:], ACTF.Ln, bias=1.0)
            nc.sync.dma_start(
                outT_d[:, gg * npr * 128 : (gg + 1) * npr * 128], ob[:]
            )

        pending = []  # (gg, ps) epilogues deferred one group for overlap
        for gg in range(g.GG):
            if gg + 1 < g.GG:
                prep_normg(gg + 1)
            T = int(TG[gg])
            ps = psG.tile([128, CW], F32, tag="ps")
            npieces = len(plan["pieces"][gg])
            for qi, (tg0, SQ, off) in enumerate(plan["pieces"][gg]):
                ne = 64 * SQ * g.CPG
                xt = xp.tile([128, g.S * g.D * g.CPG], xdt, tag="xt")
                nc.sync.dma_start(xt[:, :ne], xg_d[:, off : off + ne])
                xs = sp.tile([128, g.S * g.D * g.CPG], F16, tag="xs")
                xt_v = xt[:, :ne].rearrange("p (f t j) -> p f t j", f=g.D, j=g.CPG)
                xs_v = xs[:, :ne].rearrange("p (f t j) -> p f t j", f=g.D, j=g.CPG)

                # convert + src-norm multiply, split across engines
                c0 = 0
                while c0 < SQ:
                    CCq = min(g.CC, SQ - c0)
                    nbc = normg_v[:, None, tg0 + c0 : tg0 + c0 + CCq, :].broadcast_to(
                        [128, g.D, CCq, g.CPG]
                    )
                    o = xs_v[:, :, c0 : c0 + CCq, :]
                    i = xt_v[:, :, c0 : c0 + CCq, :]
                    eng = pattern[pat_i % len(pattern)]
                    pat_i += 1
                    if g.payload != "f8e3":
                        eng = "dve"  # f16 payload: DVE 2x handles everything
                    if eng == "pool":
                        if g.pool_mode == "tt8":
                            nc.gpsimd.tensor_tensor(o, i, nbc, ALU.mult)
                        elif g.pool_mode == "cp8":
                            nc.gpsimd.tensor_copy(o, i)
                            nc.vector.tensor_tensor(o, o, nbc, ALU.mult)
                        else:  # tt16: ACT converts, Pool multiplies
                            nc.scalar.copy(o, i)
                            nc.gpsimd.tensor_tensor(o, o, nbc, ALU.mult)
                    elif eng == "act":
                        nc.scalar.copy(o, i)
                        nc.vector.tensor_tensor(o, o, nbc, ALU.mult)
                    else:
                        nc.vector.tensor_tensor(o, i, nbc, ALU.mult)
                    c0 += CCq

                # aggregate: identity matmuls, MT tiles per instruction
                xs_t = xs[:, :ne].rearrange("p (f t j) -> p t f j", f=g.D, j=g.CPG)
                t0 = 0
                while t0 < SQ:
                    MTq = min(g.MT, SQ - t0)
                    first = qi == 0 and t0 == 0
                    last = qi == npieces - 1 and t0 + MTq == SQ
                    if MTq == 1:
                        rhs = xs_t[:, t0, :, :]
                        out_ap = ps[:]
                    else:
                        rhs = xs_t[:, t0 : t0 + MTq, :, :]
                        out_ap = ps[:, None, :].broadcast_to([128, MTq, CW])
                    nc.tensor.matmul(out_ap, ident16[:], rhs,
                                     start=first, stop=last)
                    t0 += MTq

            # defer this group's epilogue until after the next group's
            # payload work, so the DVE/PE queues never stall on psum
            pending.append((gg, ps))
            if len(pending) > 1:
                epilogue(*pending.pop(0))
        for e in pending:
            epilogue(*e)

    nc.compile()
    return nc


def _in_maps(x, weight, bias, geom, plan):
    g = geom
    x = np.ascontiguousarray(np.asarray(x, dtype=np.float32))
    deg_full = plan["deg_full"]
    xdt = _np_payload_dtype(g)
    xq = x.astype(xdt)
    xq[deg_full == 0] = 0  # src-side norm is 0 for zero-in-degree nodes
    degmax = np.maximum(deg_full, 1).astype(np.float16)

    TOT = plan["TOT"]
    w16 = np.asarray(weight, dtype=np.float32).astype(np.float16)
    wblk = np.zeros((128, 128), dtype=np.float16)
    wblk[: g.D, : g.D] = w16
    wblk[g.D :, g.D :] = w16
    bias2 = np.tile(np.asarray(bias, dtype=np.float32).reshape(g.D, 1), (2, 1))
    base = {
        "wblk": np.ascontiguousarray(wblk),
        "bias2": np.ascontiguousarray(bias2),
    }
    maps = []
    for c in range(g.CORES):
        p = plan["plans"][c]
        A = np.zeros((TOT * 128, g.CPG, g.D), dtype=xdt)
        A[p["rows"], p["j4"]] = xq[p["es"]]
        D2 = np.ones((TOT * 128, g.CPG), dtype=np.float16)
        D2[p["rows"], p["j4"]] = degmax[p["es"]]
        degA = np.ones(g.SLOTS, dtype=np.float32)
        degA[: g.NSH] = np.maximum(deg_full[c * g.NSH + p["perm"]], 1)

        # feature-outer piece-major payload: [128, f, t, j] per piece
        F = A.reshape(TOT, 128, g.CPG, g.D).transpose(1, 3, 0, 2)  # p f t j
        blocks = []
        for gl in plan["pieces"]:
            for (tg0, SQ, off) in gl:
                blocks.append(
                    np.ascontiguousarray(F[:, :, tg0 : tg0 + SQ, :]).reshape(128, -1)
                )
        xg = np.concatenate(blocks, axis=1)
        assert xg.shape[1] == plan["total_elems"]

        degg_pm = np.ascontiguousarray(
            D2.reshape(TOT, 128, g.CPG).transpose(1, 0, 2).reshape(128, -1)
        )
        maps.append(
            dict(
                base,
                xg=xg,
                degg=degg_pm,
                degA=np.ascontiguousarray(degA.reshape(g.CHPAD, 128).T),
            )
        )
    return maps


def _unshard(outTs, geom, plan):
    g = geom
    out = np.empty((g.N, g.D), dtype=np.float32)
    for c in range(g.CORES):
        perm = plan["plans"][c]["perm"]
        # outT rows (chunk%2, feat); cols (group, pair, dst)
        O = outTs[c].reshape(2, g.D, g.GG, g.CPG // 2, 128)
        C = O.transpose(2, 3, 0, 1, 4).reshape(g.CHPAD, g.D, 128)
        full = C.transpose(1, 0, 2).reshape(g.D, g.CHPAD * 128)
        out[c * g.NSH + perm] = full[:, : g.NSH].T
    return out


def run_sim(inputs, geom):
    from concourse.bass_interp import MultiCoreSim

    plan = make_plan(np.asarray(inputs["src"]), np.asarray(inputs["dst"]), geom)
    nc = build_nc(geom, plan)
    maps = _in_maps(inputs["x"], inputs["weight"], inputs["bias"], geom, plan)
    sim = MultiCoreSim(nc, num_cores=geom.CORES, trace=False)
    cores = list(sim.cores.values())
    for c, core in enumerate(cores):
        for name, arr in maps[c].items():
            core.tensor(name)[:] = arr
    sim.simulate(check_with_hw=False)
    outTs = [np.array(core.tensor("outT")) for core in cores]
    return _unshard(outTs, geom, plan)


def _install_ntff_hook():
    """The agent image's antenv lacks axon_hooks; recreate the ctypes NTFF
    profile hook (mirrors trn_agent_boot) so trace=True yields exec times."""
    import contextlib
    import ctypes
    import types

    import antenv

    if "antenv.axon_hooks" in sys.modules:
        return
    lib = ctypes.CDLL("/opt/axon/libaxon_pjrt.so")
    if not hasattr(lib, "axon_start_nrt_profile"):
        return
    lib.axon_start_nrt_profile.argtypes = [ctypes.POINTER(ctypes.c_int64), ctypes.c_size_t]
    lib.axon_start_nrt_profile.restype = ctypes.c_int64
    lib.axon_stop_nrt_profile.argtypes = [ctypes.c_char_p]
    lib.axon_stop_nrt_profile.restype = ctypes.c_int64

    @contextlib.contextmanager
    def _hook(output_dir, device_ids):
        import jax

        jax.devices()
        if device_ids:
            ids = (ctypes.c_int64 * len(device_ids))(*device_ids)
            rc = lib.axon_start_nrt_profile(ids, len(device_ids))
        else:
            rc = lib.axon_start_nrt_profile(None, 0)
        if rc != 0:
            raise RuntimeError(f"axon_start_nrt_profile rc={rc}")
        try:
            yield
        finally:
            n = lib.axon_stop_nrt_profile(str(output_dir).encode())
            print(f"ntff profile: {n} file(s) -> {output_dir}", file=sys.stderr)

    mod = types.ModuleType("antenv.axon_hooks")
    mod._hook = _hook
    mod.get_axon_ntff_profile_hook = lambda: _hook
    mod.set_axon_ntff_profile_hook = lambda h: None
    sys.modules["antenv.axon_hooks"] = mod
    antenv.axon_hooks = mod


def run_hw(inputs, geom, trace=False):
    from concourse.bass_utils import run_bass_kernel_spmd

    if trace:
        import concourse.bass_utils as _bu

        _install_ntff_hook()
        _bu.upload_artifacts = lambda d: "local://" + str(d)

    plan = make_plan(np.asarray(inputs["src"]), np.asarray(inputs["dst"]), geom)
    nc = build_nc(geom, plan)
    maps = _in_maps(inputs["x"], inputs["weight"], inputs["bias"], geom, plan)
    import tempfile

    tdir = tempfile.mkdtemp(prefix="gcde_trace_") if trace else None
    res = run_bass_kernel_spmd(
        nc, maps, core_ids=list(range(geom.CORES)), trace=trace, tmpdir=tdir
    )
    if trace:
        print("trace dir:", tdir, file=sys.stderr)
    outTs = [r["outT"] for r in res.results]
    out = _unshard(outTs, geom, plan)
    return out, res


def kernel(**inputs):
    geom = Geom(n_nodes=50000, n_cores=8)
    out, _ = run_hw(inputs, geom)
    return out
